# revision 11
# baseline (speedup 1.0000x reference)
"""LSTM warmup + autoregressive decode kernel for Trainium2 (Bass/Tile).

Reference computation (per batch row):
  h,c = 0
  for t in range(T):  h,c = LSTMstep(x_t)        # warmup over input seq
  pred0 = h @ Wd + bd
  for d in range(out_steps-1): h,c = LSTMstep(pred_d); pred_{d+1} = h@Wd+bd
  out[b, s, f] = pred_s

Strategy: data-parallel over 8 NeuronCores (B=4096 -> 512/core); the
sequential time loop is local per shard.  Everything on-chip is kept in a
*transposed* layout (partitions = unit/feature index, free dim = batch):
  z^T[1024, 512] per step via fp32r matmuls (W/U stationary, x^T/h^T moving),
  gates as [128, 2*512] tiles, so h^T feeds the next step's matmuls directly
  and the recurrence needs no transposes.  PE transposes (via identity) are
  used only to stage x^T from the input layout and to emit the output layout.
"""

import sys

for _p in ("/opt/trn_rl_repo", "/root/.axon_site/_ro/trn_rl_repo"):
    if _p not in sys.path:
        sys.path.insert(0, _p)

import numpy as np

import concourse.bacc as bacc
import concourse.mybir as mybir
import concourse.tile as tile
from concourse import bass_utils

F32 = mybir.dt.float32
F32R = mybir.dt.float32r
AF = mybir.ActivationFunctionType

N_CORES = 8
F = 64          # input/output feature dim
U = 256         # lstm units
U4 = 4 * U      # gate rows
# gate index -> position in the 1024-row z layout (keras order i,f,g,o)
G_I, G_F, G_G, G_O = 0, 1, 2, 3


def build_program(B, T, out_steps, b_nonzero, bd_nonzero, use_f32r=True):
    """Build the single-core SPMD program for a batch shard of size B."""
    assert B % 128 == 0
    NB = B // 128          # batch chunks of 128
    assert T % 2 == 0
    n_in_pairs = T // 2
    n_out_pairs = (out_steps + 1) // 2

    nc = bacc.Bacc("TRN2", target_bir_lowering=False, debug=False, num_devices=1)

    WDT = F32R if use_f32r else F32
    xin = nc.dram_tensor("xin", [B, T, F], F32, kind="ExternalInput").ap()
    w2d = nc.dram_tensor("w2", [128, U4], WDT, kind="ExternalInput").ap()
    u2d = nc.dram_tensor("u2", [128, 2 * U4], WDT, kind="ExternalInput").ap()
    wdd_d = nc.dram_tensor("wdd", [128, 2 * F], WDT, kind="ExternalInput").ap()
    ident_d = nc.dram_tensor("ident", [128, 128], F32, kind="ExternalInput").ap()
    bias8_d = nc.dram_tensor("bias8", [128, 8], F32, kind="ExternalInput").ap()
    bdup_d = nc.dram_tensor("bdup", [128, 1], F32, kind="ExternalInput").ap()
    yout = nc.dram_tensor("yout", [B, out_steps, F], F32, kind="ExternalOutput").ap()

    xin_f = xin.rearrange("b t f -> b (t f)")
    yout_f = yout.rearrange("b s f -> b (s f)")

    def mmt(ap):
        return ap.bitcast(F32R) if use_f32r else ap

    # producers feeding fp32r matmuls must declare fp32r (rounded) outputs
    rnd = mmt

    with tile.TileContext(nc) as tc:
        import contextlib

        with contextlib.ExitStack() as ctx:
            wpool = ctx.enter_context(tc.tile_pool(name="wpool", bufs=1))
            dpool = ctx.enter_context(tc.tile_pool(name="dpool", bufs=8))
            xpool = ctx.enter_context(tc.tile_pool(name="xpool", bufs=6))
            gpool = ctx.enter_context(tc.tile_pool(name="gpool", bufs=2))
            opool = ctx.enter_context(tc.tile_pool(name="opool", bufs=2))
            prpool = ctx.enter_context(tc.tile_pool(name="prpool", bufs=out_steps))
            zpool = ctx.enter_context(tc.tile_pool(name="zpool", bufs=3, space="PSUM"))
            upool = ctx.enter_context(tc.tile_pool(name="upool", bufs=2, space="PSUM"))

            # ---- constants / weights ----
            w2 = wpool.tile([128, U4], WDT)        # W duplicated rows 0:64 / 64:128
            nc.sync.dma_start(w2[:], w2d[:])
            u2 = wpool.tile([128, 2 * U4], WDT)    # U k-chunks side by side
            nc.sync.dma_start(u2[:], u2d[:])
            wdd = wpool.tile([128, 2 * F], WDT)    # Wd k-chunks side by side
            nc.sync.dma_start(wdd[:], wdd_d[:])
            ident = wpool.tile([128, 128], F32)
            nc.sync.dma_start(ident[:], ident_d[:])
            bias8 = wpool.tile([128, 8], F32)
            nc.sync.dma_start(bias8[:], bias8_d[:])
            bdup = wpool.tile([128, 1], F32)
            nc.sync.dma_start(bdup[:], bdup_d[:])

            xpairs = {}     # pair idx -> SBUF [128, B] tile of x^T for steps 2p,2p+1
            preds = {}      # decode step d -> SBUF [64, B] tile of pred_d^T
            copy_flip = [0]

            def pcopy(dst, src):
                # alternate PSUM->SBUF copies between ACT and DVE
                copy_flip[0] ^= 1
                if copy_flip[0]:
                    nc.scalar.copy(dst, src)
                else:
                    nc.vector.tensor_copy(dst, src)

            def emit_in_pair(p):
                # load inputs[:, 2p:2p+2, :] and transpose to x^T pair tile
                xp = upool.tile([128, B], F32, tag="util", name=f"xtp{p}")
                for bc in range(NB):
                    dt_in = dpool.tile([128, 128], F32, tag="din", name=f"din{p}_{bc}")
                    nc.sync.dma_start(
                        dt_in[:], xin_f[128 * bc : 128 * (bc + 1), 128 * p : 128 * (p + 1)]
                    )
                    nc.tensor.transpose(
                        xp[:, 128 * bc : 128 * (bc + 1)], dt_in[:], ident[:]
                    )
                xs = xpool.tile([128, B], F32, tag="xpair", name=f"xpair{p}")
                pcopy(rnd(xs[:]), xp[:])
                xpairs[p] = xs

            def lstm_step(t, x_src, rb, h_prev, c_prev):
                """One LSTM step. x_src: SBUF tile whose rows rb:rb+64 hold x^T.
                Returns (h, c) tiles [128, 2*B] in (uchunk, batch) layout."""
                zt = {}
                for q in (G_G, G_I, G_F, G_O):
                    zq = zpool.tile([128, 2 * B], F32, tag="z", name=f"z{t}_{q}")
                    for ch in (0, 1):
                        mcol = 256 * q + 128 * ch
                        out_ap = zq[:, B * ch : B * (ch + 1)]
                        nc.tensor.matmul(
                            out_ap,
                            mmt(w2[rb : rb + 64, mcol : mcol + 128]),
                            mmt(x_src[rb : rb + 64, :]),
                            start=True,
                            stop=(h_prev is None),
                        )
                        if h_prev is not None:
                            nc.tensor.matmul(
                                out_ap,
                                mmt(u2[:, mcol : mcol + 128]),
                                mmt(h_prev[:, 0:B]),
                                start=False,
                                stop=False,
                            )
                            nc.tensor.matmul(
                                out_ap,
                                mmt(u2[:, U4 + mcol : U4 + mcol + 128]),
                                mmt(h_prev[:, B : 2 * B]),
                                start=False,
                                stop=True,
                            )
                    zt[q] = zq

                gates = {}
                for q, func, tg in (
                    (G_G, AF.Tanh, "gg"),
                    (G_I, AF.Sigmoid, "gi"),
                    (G_F, AF.Sigmoid, "gf"),
                    (G_O, AF.Sigmoid, "go"),
                ):
                    gt = gpool.tile([128, 2 * B], F32, tag=tg, name=f"g{t}_{q}")
                    if b_nonzero:
                        for ch in (0, 1):
                            nc.scalar.activation(
                                gt[:, B * ch : B * (ch + 1)],
                                zt[q][:, B * ch : B * (ch + 1)],
                                func,
                                bias=bias8[:, 2 * q + ch : 2 * q + ch + 1],
                            )
                    else:
                        nc.scalar.activation(gt[:], zt[q][:], func)
                    gates[q] = gt

                c_t = gpool.tile([128, 2 * B], F32, tag="c", name=f"c{t}")
                if c_prev is None:
                    nc.vector.tensor_mul(c_t[:], gates[G_I][:], gates[G_G][:])
                else:
                    m2 = gpool.tile([128, 2 * B], F32, tag="m2", name=f"m2_{t}")
                    nc.vector.tensor_mul(m2[:], gates[G_I][:], gates[G_G][:])
                    fc = gpool.tile([128, 2 * B], F32, tag="fc", name=f"fc{t}")
                    nc.vector.tensor_mul(fc[:], gates[G_F][:], c_prev[:])
                    nc.vector.tensor_add(c_t[:], fc[:], m2[:])
                tc_t = gpool.tile([128, 2 * B], F32, tag="tc", name=f"tc{t}")
                nc.scalar.activation(tc_t[:], c_t[:], AF.Tanh)
                h_t = gpool.tile([128, 2 * B], F32, tag="h", name=f"h{t}")
                nc.vector.tensor_mul(rnd(h_t[:]), gates[G_O][:], tc_t[:])
                return h_t, c_t

            def emit_pred(d, h_t):
                """pred_d^T = Wd^T h + bd -> [64, B] SBUF tile at base partition 0."""
                pp = upool.tile([64, B], F32, tag="util", name=f"predp{d}")
                nc.tensor.matmul(
                    pp[:], mmt(wdd[:, 0:F]), mmt(h_t[:, 0:B]), start=True, stop=False
                )
                nc.tensor.matmul(
                    pp[:],
                    mmt(wdd[:, F : 2 * F]),
                    mmt(h_t[:, B : 2 * B]),
                    start=False,
                    stop=True,
                )
                ps = prpool.tile([64, B], F32, tag="prp", name=f"prsb{d}")
                if bd_nonzero:
                    nc.scalar.activation(
                        rnd(ps[:]), pp[:], AF.Identity, bias=bdup[0:64, 0:1]
                    )
                else:
                    pcopy(rnd(ps[:]), pp[:])
                preds[d] = ps
                return ps

            def emit_out_step(d):
                """Transpose pred_d back to [batch, feat] layout and DMA out."""
                ps = preds[d]
                tp = upool.tile([128, NB * F], F32, tag="util", name=f"otp{d}")
                for bc in range(NB):
                    nc.tensor.matmul(
                        tp[:, F * bc : F * (bc + 1)],
                        ps[:, 128 * bc : 128 * (bc + 1)],
                        ident[0:64, 0:F],
                        is_transpose=True,
                    )
                osb = opool.tile([128, NB * F], F32, tag="ot", name=f"osb{d}")
                pcopy(osb[:], tp[:])
                for bc in range(NB):
                    nc.sync.dma_start(
                        yout_f[128 * bc : 128 * (bc + 1), F * d : F * (d + 1)],
                        osb[:, F * bc : F * bc + F],
                    )
                del preds[d]

            # ---- warmup over the input sequence ----
            emit_in_pair(0)
            if n_in_pairs > 1:
                emit_in_pair(1)
            h_t = c_t = None
            for t in range(T):
                p, rb = t // 2, 64 * (t % 2)
                if t % 2 == 0 and p + 2 < n_in_pairs:
                    emit_in_pair(p + 2)
                h_t, c_t = lstm_step(t, xpairs[p], rb, h_t, c_t)
                if t % 2 == 1:
                    del xpairs[p]

            # ---- autoregressive decode ----
            emit_pred(0, h_t)
            for d in range(1, out_steps):
                h_t, c_t = lstm_step(T + d - 1, preds[d - 1], 0, h_t, c_t)
                emit_pred(d, h_t)
                emit_out_step(d - 1)
            emit_out_step(out_steps - 1)

    nc.compile()
    return nc


_CACHE = {}


def _get_program(key):
    if key not in _CACHE:
        _CACHE[key] = build_program(*key)
    return _CACHE[key]


def _host_prep(W, Uk, b, Wd, bd):
    w2 = np.concatenate([W, W], axis=0).astype(np.float32)          # [128, 1024]
    u2 = np.concatenate([Uk[0:128], Uk[128:256]], axis=1).astype(np.float32)
    wdd = np.concatenate([Wd[0:128], Wd[128:256]], axis=1).astype(np.float32)
    ident = np.eye(128, dtype=np.float32)
    bias8 = np.ascontiguousarray(b.reshape(8, 128).T.astype(np.float32))
    bdup = np.concatenate([bd, bd]).reshape(128, 1).astype(np.float32)
    return w2, u2, wdd, ident, bias8, bdup


def kernel(inputs, W, U, b, Wd, bd, out_steps):
    inputs = np.asarray(inputs, dtype=np.float32)
    W = np.asarray(W, dtype=np.float32)
    U_ = np.asarray(U, dtype=np.float32)
    b_ = np.asarray(b, dtype=np.float32)
    Wd = np.asarray(Wd, dtype=np.float32)
    bd = np.asarray(bd, dtype=np.float32)
    out_steps = int(out_steps)

    B_full, T, F_ = inputs.shape
    assert B_full % N_CORES == 0
    Bc = B_full // N_CORES
    b_nz = bool(np.any(b_ != 0))
    bd_nz = bool(np.any(bd != 0))

    nc = _get_program((Bc, T, out_steps, b_nz, bd_nz, True))
    w2, u2, wdd, ident, bias8, bdup = _host_prep(W, U_, b_, Wd, bd)

    shared = {
        "w2": w2,
        "u2": u2,
        "wdd": wdd,
        "ident": ident,
        "bias8": bias8,
        "bdup": bdup,
    }
    in_maps = [
        {"xin": np.ascontiguousarray(inputs[i * Bc : (i + 1) * Bc]), **shared}
        for i in range(N_CORES)
    ]
    res = bass_utils.run_bass_kernel_spmd(nc, in_maps, core_ids=list(range(N_CORES)))
    out = np.concatenate([res.results[i]["yout"] for i in range(N_CORES)], axis=0)
    return out
